# revision 1
# baseline (speedup 1.0000x reference)
"""Causal self-attention with RoPE — Trainium2 Bass kernel.

Problem: B=8, T=1024, C=768, H=12, D=64; y = proj(softmax(causal(rope(q)·rope(k)))·v)

Sharding: data-parallel over batch — core b computes batch element b end-to-end.
No collectives.

Per-core dataflow (all layouts chosen so no on-chip transposes are needed):
  host:  xT[c,t], w_qkvT[c,j], w_projT[c,c'] (pre-transposed, bf16)
  mm1a:  qkT[j,t] = w_qkvT.T @ xT          (j = first 1536 cols: q,k)
  rope:  qkT_roped = qkT*CC + (Pswap@qkT)*SS   (CC/SS host tables in [d,t] layout)
  mm1b:  v[t,j]  = xT.T @ w_qkvT[:,1536:]  (+ ones column per head -> v_aug)
  attn:  per head h: sT[s,t] = kT.T @ qT (seg-tiled causally), p = exp(sT/8) (*mask
         on diagonal 128-blocks), yT_aug[d+1,t] += v_aug.T @ p  (row 64 = softmax sums)
  norm:  inv = 1/sums (batched reciprocal), yT_norm = yT * bcast(inv) (PE K=1 bcast)
  proj:  out[t,c'] = yT_norm.T @ w_projT
"""

import sys

sys.path.insert(0, "/opt/trn_rl_repo")

import numpy as np
import ml_dtypes

BF16 = ml_dtypes.bfloat16

B, T, C, H = 8, 1024, 768, 12
D = C // H  # 64
NT = T // 128  # 8 t-tiles
NCT = C // 128  # 6 c-tiles
NQK = 2 * C // 128  # 12 qk row tiles

_CACHE = {}


def _host_tables():
    inv_freq = 1.0 / (10000.0 ** (np.arange(0, D, 2, dtype=np.float64) / D))  # [32]
    freqs = np.outer(np.arange(T, dtype=np.float64), inv_freq)  # [T, 32]
    cos = np.cos(freqs).astype(np.float32)  # [T, 32]
    sin = np.sin(freqs).astype(np.float32)
    cos_t = cos.T  # [32, T]
    sin_t = sin.T
    cc = np.concatenate([cos_t, cos_t, cos_t, cos_t], axis=0)  # [128, T]
    ss = np.concatenate([sin_t, -sin_t, sin_t, -sin_t], axis=0)  # [128, T]
    # Pswap (symmetric): within each 64-block swap halves; lhsT = Pswap
    blk = np.zeros((64, 64), np.float32)
    blk[:32, 32:] = np.eye(32)
    blk[32:, :32] = np.eye(32)
    pswap = np.zeros((128, 128), np.float32)
    pswap[:64, :64] = blk
    pswap[64:, 64:] = blk
    # causal keep-mask for diagonal blocks (s<=t keeps), replicated 8x along
    # the free dim so one DVE op masks all 8 diagonal blocks of a head
    m01 = (np.arange(128)[:, None] <= np.arange(128)[None, :]).astype(np.float32)
    m01r = np.tile(m01, (1, 8))
    return cc, ss, pswap, m01r


def _segs(i):
    """Causal t-segments for s-tile i: list of (t0, width). Each within one
    512-col psum bank; first 128 cols of the first seg are the diagonal block."""
    s0 = i * 128
    out = []
    if s0 < 512:
        out.append((s0, 512 - s0))
        out.append((512, 512))
    else:
        out.append((s0, 1024 - s0))
    return out


def _build_nc(stage=99):
    import bass_rust
    from concourse import bass, mybir, tile
    from concourse.vector_clock import ScopedClock

    f32 = mybir.dt.float32
    bf16 = mybir.dt.bfloat16
    EXP = mybir.ActivationFunctionType.Exp

    def split_multiwaits(nc):
        """Two walrus compat fixes: (a) at most one sem wait per instruction —
        hoist extra waits onto preceding same-engine NoOps; (b) the
        EVENT_SEMAPHORE_RANGE_CLEAR InstISA is rejected ("ISA wrong length") —
        replace it with per-sem compensating decrements computed from the
        program's total updates per semaphore."""
        import re

        totals, names = {}, {}
        for f in nc.m.functions:
            for blk in f.blocks:
                for inst in blk.instructions:
                    si = inst.sync_info
                    if si is None:
                        continue
                    for u in si.on_update:
                        assert u.update_reg is None
                        totals[u.id] = totals.get(u.id, 0) + (u.update_value or 1)
                        names[u.id] = u.ant_name
        n = 0
        for f in nc.m.functions:
            for blk in f.blocks:
                new = []
                for inst in blk.instructions:
                    si = inst.sync_info
                    if si is not None and len(si.on_wait) > 1:
                        waits = list(si.on_wait)
                        for w in waits[:-1]:
                            n += 1
                            new.append(
                                mybir.InstNoOp(
                                    name=f"{inst.name}-sw{n}",
                                    engine=inst.engine,
                                    sync_info=bass_rust.SyncInfo(
                                        on_wait=[w], on_update=[]
                                    ),
                                )
                            )
                        inst.sync_info = bass_rust.SyncInfo(
                            on_wait=[waits[-1]], on_update=list(si.on_update)
                        )
                    new.append(inst)
                blk.instructions = new

    nc = bass.Bass()
    xt_d = nc.declare_dram_parameter("xt", [C, T], bf16, isOutput=False)
    wq_d = nc.declare_dram_parameter("wqkvt", [C, 3 * C], bf16, isOutput=False)
    wp_d = nc.declare_dram_parameter("wprojt", [C, C], bf16, isOutput=False)
    cc_d = nc.declare_dram_parameter("cc", [128, T], bf16, isOutput=False)
    ss_d = nc.declare_dram_parameter("ss", [128, T], bf16, isOutput=False)
    psw_d = nc.declare_dram_parameter("pswap", [128, 128], bf16, isOutput=False)
    m01_d = nc.declare_dram_parameter("m01", [128, 8 * 128], bf16, isOutput=False)
    y_d = nc.declare_dram_parameter("y", [T, C], f32, isOutput=True)

    with tile.TileContext(nc) as tc:
        with (
            tc.tile_pool(name="persist", bufs=1) as persist,
            tc.tile_pool(name="tmp", bufs=4) as tmp,
            tc.tile_pool(name="ppool", bufs=2) as ppool,
            tc.tile_pool(name="outp", bufs=2) as outp,
            tc.tile_pool(name="psmm", bufs=3, space="PSUM") as psmm,
            tc.tile_pool(name="psy", bufs=1, space="PSUM") as psy,
        ):
            # ---- persistent SBUF residents + input DMA ----
            wq_sb = [persist.tile([128, 3 * C], bf16, tag=f"wq{i}", name=f"wq{i}") for i in range(NCT)]
            xt_sb = [persist.tile([128, T], bf16, tag=f"xt{i}", name=f"xt{i}") for i in range(NCT)]
            wp_sb = [persist.tile([128, C], bf16, tag=f"wp{i}", name=f"wp{i}") for i in range(NCT)]
            cc_sb = persist.tile([128, T], bf16, tag="cc")
            ss_sb = persist.tile([128, T], bf16, tag="ss")
            psw_sb = persist.tile([128, 128], bf16, tag="psw")
            m01_sb = persist.tile([128, 8 * 128], bf16, tag="m01")
            for i in range(NCT):
                nc.sync.dma_start(xt_sb[i][:], xt_d[i * 128 : (i + 1) * 128, :])
                nc.sync.dma_start(wq_sb[i][:], wq_d[i * 128 : (i + 1) * 128, :])
                nc.sync.dma_start(wp_sb[i][:], wp_d[i * 128 : (i + 1) * 128, :])
            nc.sync.dma_start(cc_sb[:], cc_d[:])
            nc.sync.dma_start(ss_sb[:], ss_d[:])
            nc.sync.dma_start(psw_sb[:], psw_d[:])
            nc.sync.dma_start(m01_sb[:], m01_d[:])

            if stage >= 1:
                qk_sb = [persist.tile([128, T], bf16, tag=f"qk{i}", name=f"qk{i}") for i in range(NQK)]
            if stage >= 2:
                v_sb = [persist.tile([128, H, D + 1], bf16, tag=f"v{i}", name=f"v{i}") for i in range(NT)]
            if stage >= 3:
                yraw_sb = [persist.tile([128, T], bf16, tag=f"yr{i}", name=f"yr{i}") for i in range(H // 2)]
                sumsb_sb = persist.tile([H, T], bf16, tag="sumsb")
            if stage >= 4:
                yn_sb = [persist.tile([128, T], bf16, tag=f"yn{i}", name=f"yn{i}") for i in range(NCT)]
                sums_sb = persist.tile([H, T], f32, tag="sums")
                inv_sb = persist.tile([H, T], f32, tag="inv")
                invb_sb = persist.tile([H, T], bf16, tag="invb")

            # ---- phases 1+2 interleaved: qk groups (DVE-heavy rope epilogue)
            # alternate with v groups (pure PE) so PE never starves ----
            def emit_qk(jt):
                for tch in range(2):
                    t0 = tch * 512
                    ps = psmm.tile([128, 512], f32, tag="mmA", name="ps")
                    for ct in range(NCT):
                        nc.tensor.matmul(
                            ps[:],
                            lhsT=wq_sb[ct][:, jt * 128 : (jt + 1) * 128],
                            rhs=xt_sb[ct][:, t0 : t0 + 512],
                            start=(ct == 0),
                            stop=(ct == NCT - 1),
                        )
                    old = tmp.tile([128, 512], bf16, tag="old", name="old", bufs=6)
                    nc.scalar.copy(old[:], ps[:])
                    bp = psy.tile([128, 512], f32, tag="yaug", name="bp")
                    nc.tensor.matmul(bp[:], lhsT=psw_sb[:], rhs=old[:])
                    t2 = tmp.tile([128, 512], bf16, tag="t2", name="t2", bufs=6)
                    nc.vector.tensor_mul(t2[:], old[:], cc_sb[:, t0 : t0 + 512])
                    t1 = tmp.tile([128, 512], bf16, tag="t1", name="t1", bufs=6)
                    nc.vector.tensor_mul(t1[:], bp[:], ss_sb[:, t0 : t0 + 512])
                    nc.vector.tensor_add(qk_sb[jt][:, t0 : t0 + 512], t1[:], t2[:])

            def emit_v(tt):
                for j0, jw, h0, nh in ((0, 512, 0, 8), (512, 256, 8, 4)):
                    ps = psmm.tile([128, 512], f32, tag="mmA", name="psv")
                    for ct in range(NCT):
                        nc.tensor.matmul(
                            ps[:, :jw],
                            lhsT=xt_sb[ct][:, tt * 128 : (tt + 1) * 128],
                            rhs=wq_sb[ct][:, 2 * C + j0 : 2 * C + j0 + jw],
                            start=(ct == 0),
                            stop=(ct == NCT - 1),
                        )
                    nc.scalar.copy(
                        v_sb[tt][:, h0 : h0 + nh, 0:D],
                        ps[:, :jw].rearrange("p (h d) -> p h d", h=nh),
                    )
                nc.gpsimd.memset(v_sb[tt][:, :, D : D + 1], 1.0)

            if stage >= 1:
                jobs = []
                qs = list(range(NQK))
                vs = list(range(NT)) if stage >= 2 else []
                order = [0, 1, None, 2, None, 3, None, 4, None, 5, None,
                         6, None, 7, None, 8, None, 9, 10, 11]
                vi = 0
                for o in order:
                    if o is None:
                        if vi < len(vs):
                            jobs.append(("v", vs[vi])); vi += 1
                    else:
                        jobs.append(("qk", o))
                while vi < len(vs):
                    jobs.append(("v", vs[vi])); vi += 1
                for kind, idx in jobs:
                    if kind == "qk":
                        emit_qk(idx)
                    else:
                        emit_v(idx)

            # ---- phase 3: attention per head ----
            for h in range(H if stage >= 3 else 0):
                qt = qk_sb[h // 2]
                kt = qk_sb[H // 2 + h // 2]
                po = (h % 2) * D
                yt = psy.tile([D + 1, T], f32, tag="yaug")
                ph = ppool.tile([128, NT, T], bf16, tag="p")
                # scores + exp per s-tile; p is stored column-shifted so the
                # diagonal block of every s-tile lands at local cols [0,128)
                for i in range(NT):
                    s0 = i * 128
                    lk = kt[po : po + D, s0 : s0 + 128]
                    sc = psmm.tile([128, T], f32, tag="mmA", name="sc")
                    for t0, w in _segs(i):
                        nc.tensor.matmul(
                            sc[:, t0 : t0 + w],
                            lhsT=lk,
                            rhs=qt[po : po + D, t0 : t0 + w],
                        )
                    nc.scalar.activation(
                        ph[:, i, 0 : T - s0], sc[:, s0:T], EXP, scale=0.125
                    )
                    # mask this s-tile's diagonal block right away so its pv
                    # matmuls don't wait on the other s-tiles' exps
                    nc.vector.tensor_mul(
                        ph[:, i, 0:128],
                        ph[:, i, 0:128],
                        m01_sb[:, i * 128 : (i + 1) * 128],
                    )
                # pv accumulation per psum bank; row D collects softmax sums
                bank_first = [True, True]
                writes = [(i, t0, w) for i in range(NT) for (t0, w) in _segs(i)]
                last_for_bank = {}
                for widx, (i, t0, w) in enumerate(writes):
                    last_for_bank[1 if t0 >= 512 else 0] = widx
                for widx, (i, t0, w) in enumerate(writes):
                    s0 = i * 128
                    b = 1 if t0 >= 512 else 0
                    nc.tensor.matmul(
                        yt[:, t0 : t0 + w],
                        lhsT=v_sb[i][:, h : h + 1, :],
                        rhs=ph[:, i, t0 - s0 : t0 - s0 + w],
                        start=bank_first[b],
                        stop=(last_for_bank[b] == widx),
                    )
                    bank_first[b] = False
                ytmp = tmp.tile([D + 1, T], bf16, tag="ytmp")
                nc.vector.tensor_copy(ytmp[:], yt[:])
                ro = (h % 2) * D
                nc.gpsimd.tensor_copy(yraw_sb[h // 2][ro : ro + D, :], ytmp[0:D, :])
                nc.sync.dma_start(sumsb_sb[h : h + 1, :], ytmp[D : D + 1, :])

            # ---- debug probes for truncated stages ----
            if stage < 4:
                yb = y_d[:].bitcast(bf16)  # [T, 2C] bf16 view of the fp32 output
                if stage == 0:
                    nc.gpsimd.dma_start(yb[0:128, 0:T], xt_sb[0][:])
                elif stage == 1:
                    nc.gpsimd.dma_start(yb[0:128, 0:T], qk_sb[0][:])
                    nc.gpsimd.dma_start(yb[128:256, 0:T], qk_sb[6][:])
                elif stage == 2:
                    nc.gpsimd.dma_start(
                        yb[0:128, 0 : H * (D + 1)], v_sb[0][:].rearrange("p h d -> p (h d)")
                    )
                elif stage == 3:
                    nc.gpsimd.dma_start(yb[0:128, 0:T], yraw_sb[0][:])
                    nc.gpsimd.dma_start(yb[130:142, 0:T], sumsb_sb[:])
            # ---- phase 3.5: normalize ----
            if stage >= 4:
                nc.vector.tensor_copy(sums_sb[:], sumsb_sb[:])
                nc.vector.reciprocal(inv_sb[:], sums_sb[:])
                nc.vector.tensor_copy(invb_sb[:], inv_sb[:])
                ones_sb = persist.tile([1, D], bf16, tag="ones")
                nc.vector.memset(ones_sb[:], 1.0)
                for h in range(H):
                    ib = tmp.tile([1, T], bf16, tag="ib")
                    nc.sync.dma_start(ib[:], invb_sb[h : h + 1, :])
                    bc = psmm.tile([D, T], f32, tag="mmA", name="bc")
                    for bch in range(2):
                        t0 = bch * 512
                        nc.tensor.matmul(
                            bc[:, t0 : t0 + 512],
                            lhsT=ones_sb[:],
                            rhs=ib[0:1, t0 : t0 + 512],
                        )
                    ro = (h % 2) * D
                    bcv = tmp.tile([128, T], bf16, tag="bcv")
                    nc.scalar.copy(bcv[ro : ro + D, :], bc[:])
                    nc.vector.tensor_mul(
                        yn_sb[h // 2][ro : ro + D, :],
                        yraw_sb[h // 2][ro : ro + D, :],
                        bcv[ro : ro + D, :],
                    )

            # ---- phase 4: out = yT_norm.T @ w_projT ----
            for tt in range(NT if stage >= 4 else 0):
                osb = outp.tile([128, C], f32, tag="osb")
                for j0, jw in ((0, 512), (512, 256)):
                    ps = psmm.tile([128, 512], f32, tag="mmA")
                    for ct in range(NCT):
                        nc.tensor.matmul(
                            ps[:, :jw],
                            lhsT=yn_sb[ct][:, tt * 128 : (tt + 1) * 128],
                            rhs=wp_sb[ct][:, j0 : j0 + jw],
                            start=(ct == 0),
                            stop=(ct == NCT - 1),
                        )
                    nc.vector.tensor_copy(osb[:, j0 : j0 + jw], ps[:, :jw])
                nc.gpsimd.dma_start(y_d[tt * 128 : (tt + 1) * 128, :], osb[:])

    split_multiwaits(nc)
    return nc


def _get_compiled():
    if "nc" not in _CACHE:
        _CACHE["nc"] = _build_nc()
        cc, ss, pswap, m01 = _host_tables()
        _CACHE["tables"] = {
            "cc": cc.astype(BF16),
            "ss": ss.astype(BF16),
            "pswap": pswap.astype(BF16),
            "m01": m01.astype(BF16),
        }
    return _CACHE["nc"], _CACHE["tables"]


def kernel(x, w_qkv, w_proj):
    from concourse.bass_utils import run_bass_kernel_spmd

    nc, tables = _get_compiled()
    x = np.asarray(x, dtype=np.float32)
    wq_t = np.ascontiguousarray(np.asarray(w_qkv, np.float32).T).astype(BF16)
    wp_t = np.ascontiguousarray(np.asarray(w_proj, np.float32).T).astype(BF16)
    in_maps = []
    for b in range(B):
        in_maps.append(
            {
                "xt": np.ascontiguousarray(x[b].T).astype(BF16),
                "wqkvt": wq_t,
                "wprojt": wp_t,
                **tables,
            }
        )
    res = run_bass_kernel_spmd(nc, in_maps, core_ids=list(range(B)))
    return np.stack([res.results[b]["y"].astype(np.float32) for b in range(B)], axis=0)

